# revision 19
# baseline (speedup 1.0000x reference)
"""Bahdanau attention kernel for Trainium2 (Bass/Tile), 8 NeuronCores.

Problem (per batch element b):
    q_proj = query[b] @ w1.T          # (LQ, H)
    k_proj = key[b]   @ w2.T          # (LK, H)
    score[q, k] = sum_h v[h] * tanh(q_proj[q, h] + k_proj[k, h])
    attn = softmax(score, axis=-1)    # output 1
    ctx  = attn @ value[b]            # output 2

Sharding: data-parallel over batch B=8 across the 8 cores (no collectives).

Algorithm: tanh expanded in an M=4 sine series (weighted LSQ offline):
    tanh(x) ~= sum_m beta_m sin(w_m x)
    sin(w(a+b)) = sin(wa)cos(wb) + cos(wa)sin(wb)
so the score is a rank-(2*M*H) matmul between per-side feature maps of the
small (H, L) projections.

v2c structure:
  * Scores accumulate TRANSPOSED (k on partitions): the context matmul
    consumes exp(score) directly as stationary -- no p transposes/copies.
  * Range reduction on DVE + PE (measured: GPSIMD tensor ops steal SBUF
    bandwidth from the DVE, and scalar_tensor_tensor only runs 1x):
        ub   = qkp*wp + C            (DVE; C = 1.5*2^23 magic rounding)
        kneg = -ub + C               (DVE; = -round(wp*qkp), bf16-exact)
        rc   = dgw_m @ qkp + I @ kneg  (PE, fp32 PSUM, 4 matmuls)
    The PE has slack in the m-loop and the extra matmuls keep the HAM
    clock gate at 2.4 GHz; ACT reads rc from PSUM (faster access).
  * One activation-table switch total (Sin set -> Exp set); softmax
    normalization avoids Ln: all-ones-stationary row sums, thin PE
    transposes + narrow DVE reciprocals for per-partition inv (ctx), and
    a tensor_scalar colrep + PE transpose rebuilds the q-broadcast (attn).
  * Double-wide score tiles: one Exp covers two k-blocks.
  * Inputs arrive as merged q|w1 / k|w2 tensors; DMAs go on the sync,
    scalar and gpsimd queues ordered so projection data lands first.
"""

import numpy as np

import concourse.bass as bass
import concourse.mybir as mybir
import concourse.tile as tile
from concourse import bacc
from concourse.bass_utils import run_bass_kernel_spmd
from concourse.masks import make_identity

F32 = mybir.dt.float32
BF16 = mybir.dt.bfloat16
FP16 = mybir.dt.float16

B = 8
L = 512          # LQ == LK
D = 512          # DQ == DK == DV
H = 128
P = 128          # SBUF partitions
NDB = D // P     # 4 d-blocks
NQB = L // P     # 4 query blocks

# M=4 sine fit of tanh (offline VarPro LSQ, Gaussian weight + floor).
M_FREQ = 4
WP = np.array([0.04143295796559196, 0.13482534334604263,
               0.25438579399046574, 0.40903080256149316])
BETA = np.array([1.265185167377264, 0.37469275421608605,
                 0.13864379748266895, 0.03954341691835254])

TWO_PI = float(2 * np.pi)
PI = float(np.pi)
RND_C = float(1.5 * 2 ** 23)   # fp32 magic rounding constant
EXP_BIAS = -4.0                # constant softmax shift (cancels in normalize)

_CACHED_NC = None


def _build_nc():
    nc = bacc.Bacc("TRN2", target_bir_lowering=False, debug=False)

    # Merged inputs: qw[:, db, 0:L] = q.T tile, qw[:, db, L:L+H] = w1.T tile.
    qw = nc.dram_tensor("qw", [P, NDB, L + H], BF16, kind="ExternalInput")
    kw = nc.dram_tensor("kw", [P, NDB, L + H], BF16, kind="ExternalInput")
    val = nc.dram_tensor("val", [P, NQB, D], BF16, kind="ExternalInput")
    # vb[:, 0:M] = v[h]*beta[m]; vb[:, M:2M] = -2*v[h]*beta[m]
    vb = nc.dram_tensor("vb", [H, 2 * M_FREQ], F32, kind="ExternalInput")
    # Outputs in paired-block layout; host reassembles (and transposes attn).
    attn_d = nc.dram_tensor("attn", [2, P, 2, L], BF16, kind="ExternalOutput")
    ctx_d = nc.dram_tensor("ctx", [2, P, 2, D], BF16, kind="ExternalOutput")

    with tile.TileContext(nc) as tc:
        with (
            tc.tile_pool(name="const", bufs=1) as const,
            tc.tile_pool(name="ub", bufs=2) as ub_pool,
            tc.tile_pool(name="kn", bufs=2) as kn_pool,
            tc.tile_pool(name="sin", bufs=3) as sin_pool,
            tc.tile_pool(name="h", bufs=2) as h_pool,
            tc.tile_pool(name="h2", bufs=2) as h2_pool,
            tc.tile_pool(name="cv", bufs=2) as cv_pool,
            tc.tile_pool(name="p", bufs=2) as p_pool,
            tc.tile_pool(name="outs", bufs=6) as out_pool,
            tc.tile_pool(name="ps", bufs=4, space="PSUM") as ps_pool,
        ):
            # ---------------- prologue ----------------
            ident = const.tile([P, P], BF16)
            make_identity(nc, ident[:])
            ones_sb = const.tile([P, P], BF16)
            nc.vector.memset(ones_sb[:], 1.0)
            neg4 = const.tile([P, 1], F32)
            nc.vector.memset(neg4[:], EXP_BIAS)
            # diag(wp_m) stationaries for the PE range reduction
            dgw = const.tile([P, M_FREQ - 1, P], BF16)
            for m in range(1, M_FREQ):
                nc.vector.tensor_scalar_mul(
                    dgw[:, m - 1, :], ident[:], float(WP[m])
                )

            # PE pre-warm: HAM activity while the first DMAs land.
            warm_ps = ps_pool.tile([P, 2 * L], F32, tag="ps", name="warm_ps")
            for _ in range(8):
                nc.tensor.matmul(warm_ps[:, 0:P], ident[:], ident[:])

            # Input DMAs (never on the scalar/ACT queue beyond kw).
            qw_sb = const.tile([P, NDB, L + H], BF16)
            kw_sb = const.tile([P, NDB, L + H], BF16)
            val_sb = const.tile([P, NQB, D], BF16)
            vb_sb = const.tile([H, 2 * M_FREQ], F32)
            nc.sync.dma_start(out=qw_sb[:, 0:2, :], in_=qw[:, 0:2, :])
            nc.scalar.dma_start(out=kw_sb[:, 0:2, :], in_=kw[:, 0:2, :])
            nc.sync.dma_start(out=qw_sb[:, 2:NDB, :], in_=qw[:, 2:NDB, :])
            nc.scalar.dma_start(out=kw_sb[:, 2:NDB, :], in_=kw[:, 2:NDB, :])
            nc.gpsimd.dma_start(out=vb_sb[:], in_=vb[:, :])
            nc.gpsimd.dma_start(out=val_sb[:], in_=val[:])

            # ---------------- projections ----------------
            ps_q = ps_pool.tile([H, L], F32, tag="ps", name="ps_q")
            ps_k = ps_pool.tile([H, L], F32, tag="ps", name="ps_k")
            for db in range(NDB):
                nc.tensor.matmul(
                    ps_q[:], qw_sb[:, db, L:L + H], qw_sb[:, db, 0:L],
                    start=(db == 0), stop=(db == NDB - 1),
                )
                nc.tensor.matmul(
                    ps_k[:], kw_sb[:, db, L:L + H], kw_sb[:, db, 0:L],
                    start=(db == 0), stop=(db == NDB - 1),
                )
                if db == 1:
                    # filler gated on the first DMA chunk: bridges the HAM
                    # activity window across the db23 DMA wait.
                    nc.tensor.matmul(warm_ps[:, 0:L], ident[:],
                                     qw_sb[:, 1, 0:L])
            # Single bf16 copy of the projections: every consumer (magic
            # rounding on DVE and the PE diag matmuls) reads this, so kneg
            # and rc stay mutually consistent.
            qkp = const.tile([H, 2 * L], BF16)
            # q-half cast on ACT (idle pre-m-loop), k-half on DVE: the two
            # copies run in parallel instead of serializing on the DVE.
            nc.scalar.copy(qkp[:, 0:L], ps_q[:])
            nc.vector.tensor_copy(qkp[:, L:2 * L], ps_k[:])
            # filler: keeps PE active between projections and rc matmuls
            nc.tensor.matmul(warm_ps[:, 0:L], ident[:], qkp[:, 0:L])

            # ---------------- m-pipeline ----------------
            # Double-wide transposed score tiles: [k, q] with kb pairs.
            score01 = ps_pool.tile([P, 2 * L], F32, tag="ps", name="score01")
            score23 = ps_pool.tile([P, 2 * L], F32, tag="ps", name="score23")
            score_slice = [
                (score01, 0), (score01, 1), (score23, 0), (score23, 1)
            ]

            def emit_scores(m, sin_t, cv_t):
                # scoreT[kb][k, q] += sin_k^T cv_q + cv_k^T sin_q
                for kb in range(NQB):
                    t, half = score_slice[kb]
                    sl = slice(half * L, (half + 1) * L)
                    nc.tensor.matmul(
                        t[:, sl],
                        sin_t[:, L + kb * P:L + (kb + 1) * P],
                        cv_t[:, 0:L],
                        start=(m == 0), stop=False,
                    )
                    nc.tensor.matmul(
                        t[:, sl],
                        cv_t[:, L + kb * P:L + (kb + 1) * P],
                        sin_t[:, 0:L],
                        start=False, stop=(m == M_FREQ - 1),
                    )

            rc_ts = [None] * M_FREQ
            sin_ts = [None] * M_FREQ
            h_ts = [None] * M_FREQ

            def emit_red(m):
                # kneg = -round(wp*qkp) (bf16-exact integer);
                # rc = wp*qkp + kneg on the PE in fp32 PSUM.
                ub_t = ub_pool.tile([H, 2 * L], F32, name=f"ub{m}", tag="ub")
                nc.vector.tensor_scalar(
                    ub_t[:], qkp[:], float(WP[m]), RND_C,
                    mybir.AluOpType.mult, mybir.AluOpType.add,
                )
                kn_t = kn_pool.tile([H, 2 * L], BF16, name=f"kn{m}", tag="kn")
                nc.vector.tensor_scalar(
                    kn_t[:], ub_t[:], -1.0, RND_C,
                    mybir.AluOpType.mult, mybir.AluOpType.add,
                )
                rc_t = ps_pool.tile([H, 2 * L], F32, name=f"rc{m}", tag="ps")
                for half in range(2):
                    sl = slice(half * L, (half + 1) * L)
                    nc.tensor.matmul(
                        rc_t[:, sl], dgw[:, m - 1, :], qkp[:, sl],
                        start=True, stop=False,
                    )
                    nc.tensor.matmul(
                        rc_t[:, sl], ident[:], kn_t[:, sl],
                        start=False, stop=True,
                    )
                rc_ts[m] = rc_t

            def emit_act(m, split=False):
                s_h = PI if m > 0 else PI * float(WP[0])
                s_sin = TWO_PI if m > 0 else TWO_PI * float(WP[0])
                src = rc_ts[m] if m > 0 else qkp
                h_t = h_pool.tile([H, 2 * L], FP16, name=f"h{m}", tag="h")
                sin_t = sin_pool.tile([H, 2 * L], BF16, name=f"sin{m}",
                                      tag="sin")
                halves = ((0, L), (L, 2 * L)) if split else ((0, 2 * L),)
                for lo, hi in halves:
                    nc.scalar.activation(
                        h_t[:, lo:hi], src[:, lo:hi],
                        mybir.ActivationFunctionType.Sin, scale=s_h,
                    )
                for lo, hi in halves:
                    nc.scalar.activation(
                        sin_t[:, lo:hi], src[:, lo:hi],
                        mybir.ActivationFunctionType.Sin, scale=s_sin,
                    )
                sin_ts[m], h_ts[m] = sin_t, h_t

            def emit_tail(m):
                # cv = vb - 2 vb h^2  ( = vb*cos(w x) ), carrying v*beta.
                h2_t = h2_pool.tile([H, 2 * L], FP16, name=f"h2_{m}", tag="h2")
                nc.vector.tensor_tensor(
                    h2_t[:], h_ts[m][:], h_ts[m][:], mybir.AluOpType.mult
                )
                cv_t = cv_pool.tile([H, 2 * L], BF16, name=f"cv{m}", tag="cv")
                nc.vector.tensor_scalar(
                    cv_t[:], h2_t[:],
                    vb_sb[:, M_FREQ + m:M_FREQ + m + 1],
                    vb_sb[:, m:m + 1],
                    mybir.AluOpType.mult, mybir.AluOpType.add,
                )
                emit_scores(m, sin_ts[m], cv_t)

            # ACT: h_m then sin_m per m. DVE per cycle: [h2_m, cv_m,
            # ub_{m+1}, kneg_{m+1}] -- cv lands right after h2, the next
            # m's reduction follows.
            # rc1 AND rc2 matmuls go to the PE queue before the m0 score
            # matmuls: the m0 scores wait on the cv0 chain, and rc2 must
            # not sit behind them or ACT stalls at m2.
            emit_act(0, split=True)
            emit_red(1)
            emit_red(2)
            emit_act(1)
            emit_tail(0)
            emit_act(2)
            emit_red(3)
            emit_tail(1)
            emit_act(3)
            emit_tail(2)
            emit_tail(3)

            # ---------------- softmax + context (transposed) --------------
            sums_ps = ps_pool.tile([P, L], F32, tag="ps", name="sums_ps")
            ctx01 = ps_pool.tile([P, 2 * D], F32, tag="ps", name="ctx01")
            ctx23 = ps_pool.tile([P, 2 * D], F32, tag="ps", name="ctx23")
            ctx_slice = [(ctx01, 0), (ctx01, 1), (ctx23, 0), (ctx23, 1)]

            p01 = p_pool.tile([P, 2 * L], BF16, name="p01", tag="p")
            p23 = p_pool.tile([P, 2 * L], BF16, name="p23", tag="p")
            p_of = [(p01, 0), (p01, 1), (p23, 0), (p23, 1)]

            def emit_sums(kb):
                pt, half = p_of[kb]
                nc.tensor.matmul(
                    sums_ps[:], ones_sb[:],
                    pt[:, half * L:(half + 1) * L],
                    start=(kb == 0), stop=(kb == NQB - 1),
                )

            def emit_ctx(kb):
                pt, half = p_of[kb]
                for qb in range(NQB):
                    ct, chalf = ctx_slice[qb]
                    nc.tensor.matmul(
                        ct[:, chalf * D:(chalf + 1) * D],
                        pt[:, half * L + qb * P:half * L + (qb + 1) * P],
                        val_sb[:, kb, :],
                        start=(kb == 0), stop=(kb == NQB - 1),
                    )

            nc.scalar.activation(
                p01[:], score01[:], mybir.ActivationFunctionType.Exp,
                bias=neg4[:],
            )
            emit_sums(0)
            emit_sums(1)
            emit_ctx(0)
            emit_ctx(1)
            nc.scalar.activation(
                p23[:], score23[:], mybir.ActivationFunctionType.Exp,
                bias=neg4[:],
            )
            emit_sums(2)
            emit_sums(3)

            # Normalize without extra ACT table sets. sums rows are all
            # equal; thin transposes + narrow reciprocals give per-partition
            # inv (ctx scale); colrep + PE transpose rebuild the q-major
            # broadcast for the attn scale. The thin transposes are emitted
            # BEFORE the kb2/kb3 context matmuls so the reciprocal chain is
            # not stuck behind them on the PE queue.
            sums_sb = const.tile([P, L], BF16)
            nc.scalar.copy(sums_sb[:], sums_ps[:])  # ACT is idle post-exp
            sT_ps = ps_pool.tile([P, 2 * NQB], BF16, tag="ps", name="sT_ps")
            invT_sb = const.tile([P, NQB], F32)
            colrep = const.tile([P, L], BF16)
            invb_ps = ps_pool.tile([P, L], BF16, tag="ps", name="invb_ps")
            for qb in range(NQB):
                nc.tensor.transpose(
                    sT_ps[:, 2 * qb:2 * qb + 1],
                    sums_sb[0:1, qb * P:(qb + 1) * P],
                    ident[0:1, 0:1],
                )
            emit_ctx(2)
            emit_ctx(3)
            for qb in range(NQB):
                nc.vector.reciprocal(
                    invT_sb[:, qb:qb + 1], sT_ps[:, 2 * qb:2 * qb + 1]
                )
                nc.vector.tensor_scalar_mul(
                    colrep[:, qb * P:(qb + 1) * P], ones_sb[:],
                    invT_sb[:, qb:qb + 1],
                )
            for qb in range(NQB):
                nc.tensor.transpose(
                    invb_ps[:, qb * P:(qb + 1) * P],
                    colrep[:, qb * P:(qb + 1) * P],
                    ident[:],
                )
            inv_bc = const.tile([P, L], BF16)
            nc.vector.tensor_copy(inv_bc[:], invb_ps[:])

            attn_sbs = [
                out_pool.tile([P, 2, L], BF16, name=f"attn_sb{c}", tag="o")
                for c in range(2)
            ]
            ctx_sbs = [
                out_pool.tile([P, 2, D], BF16, name=f"ctx_sb{c}", tag="o")
                for c in range(2)
            ]
            for kb in range(NQB):
                pt, half = p_of[kb]
                nc.vector.tensor_tensor(
                    attn_sbs[kb // 2][:, kb % 2, :],
                    pt[:, half * L:(half + 1) * L], inv_bc[:],
                    mybir.AluOpType.mult,
                )
                if kb == 1:
                    nc.sync.dma_start(out=attn_d[0], in_=attn_sbs[0][:])
                if kb == 3:
                    nc.gpsimd.dma_start(out=attn_d[1], in_=attn_sbs[1][:])
            for qb in range(NQB):
                # ctx scale on the ACT engine (idle after the exps): Copy
                # with a per-partition fp32 scale.
                ct, chalf = ctx_slice[qb]
                nc.scalar.mul(
                    ctx_sbs[qb // 2][:, qb % 2, :],
                    ct[:, chalf * D:(chalf + 1) * D],
                    invT_sb[:, qb:qb + 1],
                )
                if qb == 1:
                    nc.scalar.dma_start(out=ctx_d[0], in_=ctx_sbs[0][:])
                if qb == 3:
                    nc.sync.dma_start(out=ctx_d[1], in_=ctx_sbs[1][:])

    nc.compile()
    return nc


def _get_nc():
    global _CACHED_NC
    if _CACHED_NC is None:
        _CACHED_NC = _build_nc()
    return _CACHED_NC


def _in_maps(query, key, value, w1, w2, v):
    import ml_dtypes as _md

    f = np.float32
    bf = _md.bfloat16

    def tile_rows(arr):
        # [R, C] with R = NB*P  ->  [P, NB, C]: partition-major, so each
        # SBUF partition's data is one contiguous DRAM line.
        r, c = arr.shape
        nb = r // P
        return np.ascontiguousarray(arr.reshape(nb, P, c).transpose(1, 0, 2))

    w1T = tile_rows(np.asarray(w1, dtype=f).T.astype(bf))   # [P, NDB, H]
    w2T = tile_rows(np.asarray(w2, dtype=f).T.astype(bf))
    vb1 = (np.asarray(v, dtype=np.float64)[0][:, None] * BETA[None, :])
    vbm = np.concatenate([vb1, -2.0 * vb1], axis=1).astype(f)  # [H, 2M]
    maps = []
    for b in range(B):
        qT = tile_rows(np.asarray(query[b], dtype=f).T.astype(bf))
        kT = tile_rows(np.asarray(key[b], dtype=f).T.astype(bf))
        maps.append(
            {
                "qw": np.ascontiguousarray(np.concatenate([qT, w1T], axis=2)),
                "kw": np.ascontiguousarray(np.concatenate([kT, w2T], axis=2)),
                "val": tile_rows(np.asarray(value[b], dtype=f).astype(bf)),
                "vb": vbm,
            }
        )
    return maps


def run(query, key, value, w1, w2, v, trace=False, **spmd_kwargs):
    nc = _get_nc()
    res = run_bass_kernel_spmd(
        nc,
        _in_maps(query, key, value, w1, w2, v),
        list(range(B)),
        trace=trace,
        **spmd_kwargs,
    )

    def unpack(arr):
        # [2, P, 2, L] pairs -> [512, 512] with rows (2c+j)*128 + p
        a = np.asarray(arr).astype(np.float32)
        return a.transpose(0, 2, 1, 3).reshape(L, L)

    attn = np.stack(
        [unpack(res.results[b]["attn"]).T for b in range(B)]
    )
    ctx = np.stack(
        [unpack(res.results[b]["ctx"]) for b in range(B)]
    )
    return (attn, ctx), res


def kernel(query, key, value, w1, w2, v):
    (attn, ctx), _ = run(query, key, value, w1, w2, v, trace=False)
    return (attn, ctx)


# revision 21
# speedup vs baseline: 1.0697x; 1.0697x over previous
"""Bahdanau attention kernel for Trainium2 (Bass/Tile), 8 NeuronCores.

Problem (per batch element b):
    q_proj = query[b] @ w1.T          # (LQ, H)
    k_proj = key[b]   @ w2.T          # (LK, H)
    score[q, k] = sum_h v[h] * tanh(q_proj[q, h] + k_proj[k, h])
    attn = softmax(score, axis=-1)    # output 1
    ctx  = attn @ value[b]            # output 2

Sharding: data-parallel over batch B=8 across the 8 cores (no collectives).

Algorithm: tanh expanded in an M=4 sine series (weighted LSQ offline):
    tanh(x) ~= sum_m beta_m sin(w_m x)
    sin(w(a+b)) = sin(wa)cos(wb) + cos(wa)sin(wb)
so the score is a rank-(2*M*H) matmul between per-side feature maps of the
small (H, L) projections.

v2c structure:
  * Scores accumulate TRANSPOSED (k on partitions): the context matmul
    consumes exp(score) directly as stationary -- no p transposes/copies.
  * Range reduction on DVE + PE (measured: GPSIMD tensor ops steal SBUF
    bandwidth from the DVE, and scalar_tensor_tensor only runs 1x):
        ub   = qkp*wp + C            (DVE; C = 1.5*2^23 magic rounding)
        kneg = -ub + C               (DVE; = -round(wp*qkp), bf16-exact)
        rc   = dgw_m @ qkp + I @ kneg  (PE, fp32 PSUM, 4 matmuls)
    The PE has slack in the m-loop and the extra matmuls keep the HAM
    clock gate at 2.4 GHz; ACT reads rc from PSUM (faster access).
  * One activation-table switch total (Sin set -> Exp set); softmax
    normalization avoids Ln: all-ones-stationary row sums, thin PE
    transposes + narrow DVE reciprocals for per-partition inv (ctx), and
    a tensor_scalar colrep + PE transpose rebuilds the q-broadcast (attn).
  * Double-wide score tiles: one Exp covers two k-blocks.
  * Inputs arrive as merged q|w1 / k|w2 tensors; DMAs go on the sync,
    scalar and gpsimd queues ordered so projection data lands first.
"""

import numpy as np

import concourse.bass as bass
import concourse.mybir as mybir
import concourse.tile as tile
from concourse import bacc
from concourse.bass_utils import run_bass_kernel_spmd
from concourse.masks import make_identity

F32 = mybir.dt.float32
BF16 = mybir.dt.bfloat16
FP16 = mybir.dt.float16

B = 8
L = 512          # LQ == LK
D = 512          # DQ == DK == DV
H = 128
P = 128          # SBUF partitions
NDB = D // P     # 4 d-blocks
NQB = L // P     # 4 query blocks

# M=4 sine fit of tanh (offline VarPro LSQ, Gaussian weight + floor).
M_FREQ = 4
WP = np.array([0.04143295796559196, 0.13482534334604263,
               0.25438579399046574, 0.40903080256149316])
BETA = np.array([1.265185167377264, 0.37469275421608605,
                 0.13864379748266895, 0.03954341691835254])

TWO_PI = float(2 * np.pi)
PI = float(np.pi)
RND_C = float(1.5 * 2 ** 23)   # fp32 magic rounding constant
EXP_BIAS = -4.0                # constant softmax shift (cancels in normalize)

_CACHED_NC = None


def _build_nc():
    nc = bacc.Bacc("TRN2", target_bir_lowering=False, debug=False)

    # Merged inputs: qw[:, db, 0:L] = q.T tile, qw[:, db, L:L+H] = w1.T tile.
    qw = nc.dram_tensor("qw", [P, NDB, L + H], BF16, kind="ExternalInput")
    kw = nc.dram_tensor("kw", [P, NDB, L + H], BF16, kind="ExternalInput")
    val = nc.dram_tensor("val", [P, NQB, D], BF16, kind="ExternalInput")
    # vb[:, 0:M] = v[h]*beta[m]; vb[:, M:2M] = -2*v[h]*beta[m]
    vb = nc.dram_tensor("vb", [H, 2 * M_FREQ], F32, kind="ExternalInput")
    # Outputs in paired-block layout; host reassembles (and transposes attn).
    attn_d = nc.dram_tensor("attn", [2, P, 2, L], BF16, kind="ExternalOutput")
    ctx_d = nc.dram_tensor("ctx", [2, P, 2, D], BF16, kind="ExternalOutput")

    with tile.TileContext(nc) as tc:
        with (
            tc.tile_pool(name="const", bufs=1) as const,
            tc.tile_pool(name="ub", bufs=2) as ub_pool,
            tc.tile_pool(name="kn", bufs=2) as kn_pool,
            tc.tile_pool(name="sin", bufs=3) as sin_pool,
            tc.tile_pool(name="h", bufs=2) as h_pool,
            tc.tile_pool(name="h2", bufs=2) as h2_pool,
            tc.tile_pool(name="cv", bufs=2) as cv_pool,
            tc.tile_pool(name="p", bufs=2) as p_pool,
            tc.tile_pool(name="outs", bufs=6) as out_pool,
            tc.tile_pool(name="ps", bufs=4, space="PSUM") as ps_pool,
        ):
            # ---------------- prologue ----------------
            ident = const.tile([P, P], BF16)
            make_identity(nc, ident[:])
            ones_sb = const.tile([P, P], BF16)
            nc.vector.memset(ones_sb[:], 1.0)
            neg4 = const.tile([P, 1], F32)
            nc.vector.memset(neg4[:], EXP_BIAS)
            # Dummy Sin: forces the trig table load NOW (ACT idle), so the
            # later cast_q Copy doesn't push the load into the m-loop.
            dummy = const.tile([P, 1], BF16)
            nc.scalar.activation(
                dummy[:], neg4[:], mybir.ActivationFunctionType.Sin,
                scale=0.0,
            )
            # diag(wp_m) stationaries for the PE range reduction
            dgw = const.tile([P, M_FREQ - 1, P], BF16)
            for m in range(1, M_FREQ):
                nc.vector.tensor_scalar_mul(
                    dgw[:, m - 1, :], ident[:], float(WP[m])
                )

            # PE pre-warm: HAM activity while the first DMAs land.
            warm_ps = ps_pool.tile([P, 2 * L], F32, tag="ps", name="warm_ps")
            for _ in range(8):
                nc.tensor.matmul(warm_ps[:, 0:P], ident[:], ident[:])

            # Input DMAs (never on the scalar/ACT queue beyond kw).
            qw_sb = const.tile([P, NDB, L + H], BF16)
            kw_sb = const.tile([P, NDB, L + H], BF16)
            val_sb = const.tile([P, NQB, D], BF16)
            vb_sb = const.tile([H, 2 * M_FREQ], F32)
            nc.sync.dma_start(out=qw_sb[:, 0:2, :], in_=qw[:, 0:2, :])
            nc.scalar.dma_start(out=kw_sb[:, 0:2, :], in_=kw[:, 0:2, :])
            nc.sync.dma_start(out=qw_sb[:, 2:NDB, :], in_=qw[:, 2:NDB, :])
            nc.scalar.dma_start(out=kw_sb[:, 2:NDB, :], in_=kw[:, 2:NDB, :])
            nc.gpsimd.dma_start(out=vb_sb[:], in_=vb[:, :])
            nc.gpsimd.dma_start(out=val_sb[:], in_=val[:])
            # DMA-gated warm matmuls: execute right as the first chunks
            # land, so the 3.4us HAM warm-up clock starts ticking at data
            # arrival instead of inside the projections.
            nc.tensor.matmul(warm_ps[:, 0:L], ident[:], qw_sb[:, 0, 0:L])
            nc.tensor.matmul(warm_ps[:, 0:L], ident[:], kw_sb[:, 0, 0:L])
            nc.tensor.matmul(warm_ps[:, 0:L], ident[:], qw_sb[:, 1, 0:L])

            # ---------------- projections ----------------
            ps_q = ps_pool.tile([H, L], F32, tag="ps", name="ps_q")
            ps_k = ps_pool.tile([H, L], F32, tag="ps", name="ps_k")
            for db in range(NDB):
                nc.tensor.matmul(
                    ps_q[:], qw_sb[:, db, L:L + H], qw_sb[:, db, 0:L],
                    start=(db == 0), stop=(db == NDB - 1),
                )
                nc.tensor.matmul(
                    ps_k[:], kw_sb[:, db, L:L + H], kw_sb[:, db, 0:L],
                    start=(db == 0), stop=(db == NDB - 1),
                )
                if db == 1:
                    # filler gated on the first DMA chunk: bridges the HAM
                    # activity window across the db23 DMA wait.
                    nc.tensor.matmul(warm_ps[:, 0:L], ident[:],
                                     qw_sb[:, 1, 0:L])
            # Single bf16 copy of the projections: every consumer (magic
            # rounding on DVE and the PE diag matmuls) reads this, so kneg
            # and rc stay mutually consistent.
            qkp = const.tile([H, 2 * L], BF16)
            # q-half cast on ACT (idle pre-m-loop), k-half on DVE: the two
            # copies run in parallel instead of serializing on the DVE.
            nc.scalar.copy(qkp[:, 0:L], ps_q[:])
            nc.vector.tensor_copy(qkp[:, L:2 * L], ps_k[:])
            # filler: keeps PE active between projections and rc matmuls
            nc.tensor.matmul(warm_ps[:, 0:L], ident[:], qkp[:, 0:L])

            # ---------------- m-pipeline ----------------
            # Double-wide transposed score tiles: [k, q] with kb pairs.
            score01 = ps_pool.tile([P, 2 * L], F32, tag="ps", name="score01")
            score23 = ps_pool.tile([P, 2 * L], F32, tag="ps", name="score23")
            score_slice = [
                (score01, 0), (score01, 1), (score23, 0), (score23, 1)
            ]

            def emit_scores(m, sin_t, cv_t):
                # scoreT[kb][k, q] += sin_k^T cv_q + cv_k^T sin_q
                for kb in range(NQB):
                    t, half = score_slice[kb]
                    sl = slice(half * L, (half + 1) * L)
                    nc.tensor.matmul(
                        t[:, sl],
                        sin_t[:, L + kb * P:L + (kb + 1) * P],
                        cv_t[:, 0:L],
                        start=(m == 0), stop=False,
                    )
                    nc.tensor.matmul(
                        t[:, sl],
                        cv_t[:, L + kb * P:L + (kb + 1) * P],
                        sin_t[:, 0:L],
                        start=False, stop=(m == M_FREQ - 1),
                    )

            rc_ts = [None] * M_FREQ
            sin_ts = [None] * M_FREQ
            h_ts = [None] * M_FREQ

            def emit_red(m):
                # kneg = -round(wp*qkp) (bf16-exact integer);
                # rc = wp*qkp + kneg on the PE in fp32 PSUM.
                ub_t = ub_pool.tile([H, 2 * L], F32, name=f"ub{m}", tag="ub")
                nc.vector.tensor_scalar(
                    ub_t[:], qkp[:], float(WP[m]), RND_C,
                    mybir.AluOpType.mult, mybir.AluOpType.add,
                )
                kn_t = kn_pool.tile([H, 2 * L], BF16, name=f"kn{m}", tag="kn")
                nc.vector.tensor_scalar(
                    kn_t[:], ub_t[:], -1.0, RND_C,
                    mybir.AluOpType.mult, mybir.AluOpType.add,
                )
                rc_t = ps_pool.tile([H, 2 * L], F32, name=f"rc{m}", tag="ps")
                for half in range(2):
                    sl = slice(half * L, (half + 1) * L)
                    nc.tensor.matmul(
                        rc_t[:, sl], dgw[:, m - 1, :], qkp[:, sl],
                        start=True, stop=False,
                    )
                    nc.tensor.matmul(
                        rc_t[:, sl], ident[:], kn_t[:, sl],
                        start=False, stop=True,
                    )
                rc_ts[m] = rc_t

            def emit_act(m, split=False):
                s_h = PI if m > 0 else PI * float(WP[0])
                s_sin = TWO_PI if m > 0 else TWO_PI * float(WP[0])
                src = rc_ts[m] if m > 0 else qkp
                h_t = h_pool.tile([H, 2 * L], FP16, name=f"h{m}", tag="h")
                sin_t = sin_pool.tile([H, 2 * L], BF16, name=f"sin{m}",
                                      tag="sin")
                halves = ((0, L), (L, 2 * L)) if split else ((0, 2 * L),)
                for lo, hi in halves:
                    nc.scalar.activation(
                        h_t[:, lo:hi], src[:, lo:hi],
                        mybir.ActivationFunctionType.Sin, scale=s_h,
                    )
                for lo, hi in halves:
                    nc.scalar.activation(
                        sin_t[:, lo:hi], src[:, lo:hi],
                        mybir.ActivationFunctionType.Sin, scale=s_sin,
                    )
                sin_ts[m], h_ts[m] = sin_t, h_t

            def emit_tail(m):
                # cv = vb - 2 vb h^2  ( = vb*cos(w x) ), carrying v*beta.
                h2_t = h2_pool.tile([H, 2 * L], FP16, name=f"h2_{m}", tag="h2")
                nc.vector.tensor_tensor(
                    h2_t[:], h_ts[m][:], h_ts[m][:], mybir.AluOpType.mult
                )
                cv_t = cv_pool.tile([H, 2 * L], BF16, name=f"cv{m}", tag="cv")
                nc.vector.tensor_scalar(
                    cv_t[:], h2_t[:],
                    vb_sb[:, M_FREQ + m:M_FREQ + m + 1],
                    vb_sb[:, m:m + 1],
                    mybir.AluOpType.mult, mybir.AluOpType.add,
                )
                emit_scores(m, sin_ts[m], cv_t)

            # ACT: h_m then sin_m per m. DVE per cycle: [h2_m, cv_m,
            # ub_{m+1}, kneg_{m+1}] -- cv lands right after h2, the next
            # m's reduction follows.
            # rc1 AND rc2 matmuls go to the PE queue before the m0 score
            # matmuls: the m0 scores wait on the cv0 chain, and rc2 must
            # not sit behind them or ACT stalls at m2.
            emit_act(0, split=True)
            emit_red(1)
            emit_red(2)
            emit_act(1)
            emit_tail(0)
            emit_act(2)
            emit_red(3)
            emit_tail(1)
            emit_act(3)
            emit_tail(2)
            emit_tail(3)

            # ---------------- softmax + context (transposed) --------------
            sums_ps = ps_pool.tile([P, L], F32, tag="ps", name="sums_ps")
            ctx01 = ps_pool.tile([P, 2 * D], F32, tag="ps", name="ctx01")
            ctx23 = ps_pool.tile([P, 2 * D], F32, tag="ps", name="ctx23")
            ctx_slice = [(ctx01, 0), (ctx01, 1), (ctx23, 0), (ctx23, 1)]

            p01 = p_pool.tile([P, 2 * L], BF16, name="p01", tag="p")
            p23 = p_pool.tile([P, 2 * L], BF16, name="p23", tag="p")
            p_of = [(p01, 0), (p01, 1), (p23, 0), (p23, 1)]

            def emit_sums(kb):
                pt, half = p_of[kb]
                nc.tensor.matmul(
                    sums_ps[:], ones_sb[:],
                    pt[:, half * L:(half + 1) * L],
                    start=(kb == 0), stop=(kb == NQB - 1),
                )

            def emit_ctx(kb):
                pt, half = p_of[kb]
                for qb in range(NQB):
                    ct, chalf = ctx_slice[qb]
                    nc.tensor.matmul(
                        ct[:, chalf * D:(chalf + 1) * D],
                        pt[:, half * L + qb * P:half * L + (qb + 1) * P],
                        val_sb[:, kb, :],
                        start=(kb == 0), stop=(kb == NQB - 1),
                    )

            nc.scalar.activation(
                p01[:], score01[:], mybir.ActivationFunctionType.Exp,
                bias=neg4[:],
            )
            emit_sums(0)
            emit_sums(1)
            emit_ctx(0)
            emit_ctx(1)
            nc.scalar.activation(
                p23[:], score23[:], mybir.ActivationFunctionType.Exp,
                bias=neg4[:],
            )
            emit_sums(2)
            emit_sums(3)

            # Normalize without extra ACT table sets. sums rows are all
            # equal; thin transposes + narrow reciprocals give per-partition
            # inv (ctx scale); colrep + PE transpose rebuild the q-major
            # broadcast for the attn scale. The thin transposes are emitted
            # BEFORE the kb2/kb3 context matmuls so the reciprocal chain is
            # not stuck behind them on the PE queue.
            sums_sb = const.tile([P, L], BF16)
            nc.scalar.copy(sums_sb[:], sums_ps[:])  # ACT is idle post-exp
            sT_ps = ps_pool.tile([P, 2 * NQB], BF16, tag="ps", name="sT_ps")
            invT_sb = const.tile([P, NQB], F32)
            colrep = const.tile([P, L], BF16)
            invb_ps = ps_pool.tile([P, L], BF16, tag="ps", name="invb_ps")
            for qb in range(NQB):
                nc.tensor.transpose(
                    sT_ps[:, 2 * qb:2 * qb + 1],
                    sums_sb[0:1, qb * P:(qb + 1) * P],
                    ident[0:1, 0:1],
                )
            emit_ctx(2)
            emit_ctx(3)
            for qb in range(NQB):
                nc.vector.reciprocal(
                    invT_sb[:, qb:qb + 1], sT_ps[:, 2 * qb:2 * qb + 1]
                )
                nc.vector.tensor_scalar_mul(
                    colrep[:, qb * P:(qb + 1) * P], ones_sb[:],
                    invT_sb[:, qb:qb + 1],
                )
            for qb in range(NQB):
                nc.tensor.transpose(
                    invb_ps[:, qb * P:(qb + 1) * P],
                    colrep[:, qb * P:(qb + 1) * P],
                    ident[:],
                )
            inv_bc = const.tile([P, L], BF16)
            nc.vector.tensor_copy(inv_bc[:], invb_ps[:])

            attn_sbs = [
                out_pool.tile([P, 2, L], BF16, name=f"attn_sb{c}", tag="o")
                for c in range(2)
            ]
            ctx_sbs = [
                out_pool.tile([P, 2, D], BF16, name=f"ctx_sb{c}", tag="o")
                for c in range(2)
            ]
            for kb in range(NQB):
                pt, half = p_of[kb]
                nc.vector.tensor_tensor(
                    attn_sbs[kb // 2][:, kb % 2, :],
                    pt[:, half * L:(half + 1) * L], inv_bc[:],
                    mybir.AluOpType.mult,
                )
                if kb == 1:
                    nc.sync.dma_start(out=attn_d[0], in_=attn_sbs[0][:])
                if kb == 3:
                    nc.gpsimd.dma_start(out=attn_d[1], in_=attn_sbs[1][:])
            for qb in range(NQB):
                # ctx scale on the ACT engine (idle after the exps): Copy
                # with a per-partition fp32 scale.
                ct, chalf = ctx_slice[qb]
                nc.scalar.mul(
                    ctx_sbs[qb // 2][:, qb % 2, :],
                    ct[:, chalf * D:(chalf + 1) * D],
                    invT_sb[:, qb:qb + 1],
                )
                if qb == 1:
                    nc.scalar.dma_start(out=ctx_d[0], in_=ctx_sbs[0][:])
                if qb == 3:
                    nc.sync.dma_start(out=ctx_d[1], in_=ctx_sbs[1][:])

    nc.compile()
    return nc


def _get_nc():
    global _CACHED_NC
    if _CACHED_NC is None:
        _CACHED_NC = _build_nc()
    return _CACHED_NC


def _in_maps(query, key, value, w1, w2, v):
    import ml_dtypes as _md

    f = np.float32
    bf = _md.bfloat16

    def tile_rows(arr):
        # [R, C] with R = NB*P  ->  [P, NB, C]: partition-major, so each
        # SBUF partition's data is one contiguous DRAM line.
        r, c = arr.shape
        nb = r // P
        return np.ascontiguousarray(arr.reshape(nb, P, c).transpose(1, 0, 2))

    w1T = tile_rows(np.asarray(w1, dtype=f).T.astype(bf))   # [P, NDB, H]
    w2T = tile_rows(np.asarray(w2, dtype=f).T.astype(bf))
    vb1 = (np.asarray(v, dtype=np.float64)[0][:, None] * BETA[None, :])
    vbm = np.concatenate([vb1, -2.0 * vb1], axis=1).astype(f)  # [H, 2M]
    maps = []
    for b in range(B):
        qT = tile_rows(np.asarray(query[b], dtype=f).T.astype(bf))
        kT = tile_rows(np.asarray(key[b], dtype=f).T.astype(bf))
        maps.append(
            {
                "qw": np.ascontiguousarray(np.concatenate([qT, w1T], axis=2)),
                "kw": np.ascontiguousarray(np.concatenate([kT, w2T], axis=2)),
                "val": tile_rows(np.asarray(value[b], dtype=f).astype(bf)),
                "vb": vbm,
            }
        )
    return maps


def run(query, key, value, w1, w2, v, trace=False, **spmd_kwargs):
    nc = _get_nc()
    res = run_bass_kernel_spmd(
        nc,
        _in_maps(query, key, value, w1, w2, v),
        list(range(B)),
        trace=trace,
        **spmd_kwargs,
    )

    def unpack(arr):
        # [2, P, 2, L] pairs -> [512, 512] with rows (2c+j)*128 + p
        a = np.asarray(arr).astype(np.float32)
        return a.transpose(0, 2, 1, 3).reshape(L, L)

    attn = np.stack(
        [unpack(res.results[b]["attn"]).T for b in range(B)]
    )
    ctx = np.stack(
        [unpack(res.results[b]["ctx"]) for b in range(B)]
    )
    return (attn, ctx), res


def kernel(query, key, value, w1, w2, v):
    (attn, ctx), _ = run(query, key, value, w1, w2, v, trace=False)
    return (attn, ctx)


# revision 22
# speedup vs baseline: 1.1027x; 1.0309x over previous
"""Bahdanau attention kernel for Trainium2 (Bass/Tile), 8 NeuronCores.

Problem (per batch element b):
    q_proj = query[b] @ w1.T          # (LQ, H)
    k_proj = key[b]   @ w2.T          # (LK, H)
    score[q, k] = sum_h v[h] * tanh(q_proj[q, h] + k_proj[k, h])
    attn = softmax(score, axis=-1)    # output 1
    ctx  = attn @ value[b]            # output 2

Sharding: data-parallel over batch B=8 across the 8 cores (no collectives).

Algorithm: tanh expanded in an M=4 sine series (weighted LSQ offline):
    tanh(x) ~= sum_m beta_m sin(w_m x)
    sin(w(a+b)) = sin(wa)cos(wb) + cos(wa)sin(wb)
so the score is a rank-(2*M*H) matmul between per-side feature maps of the
small (H, L) projections.

v2c structure:
  * Scores accumulate TRANSPOSED (k on partitions): the context matmul
    consumes exp(score) directly as stationary -- no p transposes/copies.
  * Range reduction on DVE + PE (measured: GPSIMD tensor ops steal SBUF
    bandwidth from the DVE, and scalar_tensor_tensor only runs 1x):
        ub   = qkp*wp + C            (DVE; C = 1.5*2^23 magic rounding)
        kneg = -ub + C               (DVE; = -round(wp*qkp), bf16-exact)
        rc   = dgw_m @ qkp + I @ kneg  (PE, fp32 PSUM, 4 matmuls)
    The PE has slack in the m-loop and the extra matmuls keep the HAM
    clock gate at 2.4 GHz; ACT reads rc from PSUM (faster access).
  * One activation-table switch total (Sin set -> Exp set); softmax
    normalization avoids Ln: all-ones-stationary row sums, thin PE
    transposes + narrow DVE reciprocals for per-partition inv (ctx), and
    a tensor_scalar colrep + PE transpose rebuilds the q-broadcast (attn).
  * Double-wide score tiles: one Exp covers two k-blocks.
  * Inputs arrive as merged q|w1 / k|w2 tensors; DMAs go on the sync,
    scalar and gpsimd queues ordered so projection data lands first.
"""

import numpy as np

import concourse.bass as bass
import concourse.mybir as mybir
import concourse.tile as tile
from concourse import bacc
from concourse.bass_utils import run_bass_kernel_spmd
from concourse.masks import make_identity

F32 = mybir.dt.float32
BF16 = mybir.dt.bfloat16
FP16 = mybir.dt.float16

B = 8
L = 512          # LQ == LK
D = 512          # DQ == DK == DV
H = 128
P = 128          # SBUF partitions
NDB = D // P     # 4 d-blocks
NQB = L // P     # 4 query blocks

# M=4 sine fit of tanh (offline VarPro LSQ, Gaussian weight + floor).
M_FREQ = 4
WP = np.array([0.04143295796559196, 0.13482534334604263,
               0.25438579399046574, 0.40903080256149316])
BETA = np.array([1.265185167377264, 0.37469275421608605,
                 0.13864379748266895, 0.03954341691835254])

TWO_PI = float(2 * np.pi)
PI = float(np.pi)
RND_C = float(1.5 * 2 ** 23)   # fp32 magic rounding constant
EXP_BIAS = -4.0                # constant softmax shift (cancels in normalize)

_CACHED_NC = None


def _build_nc():
    nc = bacc.Bacc("TRN2", target_bir_lowering=False, debug=False)

    # Merged inputs: qw[:, db, 0:L] = q.T tile, qw[:, db, L:L+H] = w1.T tile.
    qw = nc.dram_tensor("qw", [P, NDB, L + H], BF16, kind="ExternalInput")
    kw = nc.dram_tensor("kw", [P, NDB, L + H], BF16, kind="ExternalInput")
    val = nc.dram_tensor("val", [P, NQB, D], BF16, kind="ExternalInput")
    # vb[:, 0:M] = v[h]*beta[m]; vb[:, M:2M] = -2*v[h]*beta[m]
    vb = nc.dram_tensor("vb", [H, 2 * M_FREQ], F32, kind="ExternalInput")
    # Outputs in paired-block layout; host reassembles (and transposes attn).
    attn_d = nc.dram_tensor("attn", [2, P, 2, L], BF16, kind="ExternalOutput")
    ctx_d = nc.dram_tensor("ctx", [2, P, 2, D], BF16, kind="ExternalOutput")

    with tile.TileContext(nc) as tc:
        with (
            tc.tile_pool(name="const", bufs=1) as const,
            tc.tile_pool(name="ub", bufs=2) as ub_pool,
            tc.tile_pool(name="kn", bufs=2) as kn_pool,
            tc.tile_pool(name="sin", bufs=3) as sin_pool,
            tc.tile_pool(name="h", bufs=2) as h_pool,
            tc.tile_pool(name="h2", bufs=2) as h2_pool,
            tc.tile_pool(name="cv", bufs=2) as cv_pool,
            tc.tile_pool(name="p", bufs=2) as p_pool,
            tc.tile_pool(name="outs", bufs=6) as out_pool,
            tc.tile_pool(name="ps", bufs=4, space="PSUM") as ps_pool,
        ):
            # ---------------- prologue ----------------
            ident = const.tile([P, P], BF16)
            make_identity(nc, ident[:])
            ones_sb = const.tile([P, P], BF16)
            nc.vector.memset(ones_sb[:], 1.0)
            neg4 = const.tile([P, 1], F32)
            nc.vector.memset(neg4[:], EXP_BIAS)
            # Dummy Sin: forces the trig table load NOW (ACT idle), so the
            # later cast_q Copy doesn't push the load into the m-loop.
            dummy = const.tile([P, 1], BF16)
            nc.scalar.activation(
                dummy[:], neg4[:], mybir.ActivationFunctionType.Sin,
                scale=0.0,
            )
            # diag(wp_m) stationaries for the PE range reduction
            dgw = const.tile([P, M_FREQ - 1, P], BF16)
            for m in range(1, M_FREQ):
                nc.vector.tensor_scalar_mul(
                    dgw[:, m - 1, :], ident[:], float(WP[m])
                )

            # PE pre-warm: HAM activity while the first DMAs land.
            warm_ps = ps_pool.tile([P, 2 * L], F32, tag="ps", name="warm_ps")
            for _ in range(8):
                nc.tensor.matmul(warm_ps[:, 0:P], ident[:], ident[:])

            # Input DMAs (never on the scalar/ACT queue beyond kw).
            qw_sb = const.tile([P, NDB, L + H], BF16)
            kw_sb = const.tile([P, NDB, L + H], BF16)
            val_sb = const.tile([P, NQB, D], BF16)
            vb_sb = const.tile([H, 2 * M_FREQ], F32)
            nc.sync.dma_start(out=qw_sb[:, 0:2, :], in_=qw[:, 0:2, :])
            nc.scalar.dma_start(out=kw_sb[:, 0:2, :], in_=kw[:, 0:2, :])
            nc.sync.dma_start(out=qw_sb[:, 2:NDB, :], in_=qw[:, 2:NDB, :])
            nc.scalar.dma_start(out=kw_sb[:, 2:NDB, :], in_=kw[:, 2:NDB, :])
            nc.gpsimd.dma_start(out=vb_sb[:], in_=vb[:, :])
            nc.gpsimd.dma_start(out=val_sb[:], in_=val[:])
            # DMA-gated warm matmuls: execute right as the first chunks
            # land, so the 3.4us HAM warm-up clock starts ticking at data
            # arrival instead of inside the projections.
            nc.tensor.matmul(warm_ps[:, 0:L], ident[:], qw_sb[:, 0, 0:L])
            nc.tensor.matmul(warm_ps[:, 0:L], ident[:], kw_sb[:, 0, 0:L])
            nc.tensor.matmul(warm_ps[:, 0:L], ident[:], qw_sb[:, 1, 0:L])

            # ---------------- projections ----------------
            ps_q = ps_pool.tile([H, L], F32, tag="ps", name="ps_q")
            ps_k = ps_pool.tile([H, L], F32, tag="ps", name="ps_k")
            for db in range(NDB):
                nc.tensor.matmul(
                    ps_q[:], qw_sb[:, db, L:L + H], qw_sb[:, db, 0:L],
                    start=(db == 0), stop=(db == NDB - 1),
                )
                nc.tensor.matmul(
                    ps_k[:], kw_sb[:, db, L:L + H], kw_sb[:, db, 0:L],
                    start=(db == 0), stop=(db == NDB - 1),
                )
                if db == 1:
                    # filler gated on the first DMA chunk: bridges the HAM
                    # activity window across the db23 DMA wait.
                    nc.tensor.matmul(warm_ps[:, 0:L], ident[:],
                                     qw_sb[:, 1, 0:L])
            # Single bf16 copy of the projections: every consumer (magic
            # rounding on DVE and the PE diag matmuls) reads this, so kneg
            # and rc stay mutually consistent.
            qkp = const.tile([H, 2 * L], BF16)
            # q-half cast on ACT (idle pre-m-loop), k-half on DVE: the two
            # copies run in parallel instead of serializing on the DVE.
            nc.scalar.copy(qkp[:, 0:L], ps_q[:])
            nc.vector.tensor_copy(qkp[:, L:2 * L], ps_k[:])
            # filler: keeps PE active between projections and rc matmuls
            nc.tensor.matmul(warm_ps[:, 0:L], ident[:], qkp[:, 0:L])

            # ---------------- m-pipeline ----------------
            # Double-wide transposed score tiles: [k, q] with kb pairs.
            score01 = ps_pool.tile([P, 2 * L], F32, tag="ps", name="score01")
            score23 = ps_pool.tile([P, 2 * L], F32, tag="ps", name="score23")
            score_slice = [
                (score01, 0), (score01, 1), (score23, 0), (score23, 1)
            ]

            def emit_scores(m, sin_t, cv_t):
                # scoreT[kb][k, q] += sin_k^T cv_q + cv_k^T sin_q
                for kb in range(NQB):
                    t, half = score_slice[kb]
                    sl = slice(half * L, (half + 1) * L)
                    nc.tensor.matmul(
                        t[:, sl],
                        sin_t[:, L + kb * P:L + (kb + 1) * P],
                        cv_t[:, 0:L],
                        start=(m == 0), stop=False,
                    )
                    nc.tensor.matmul(
                        t[:, sl],
                        cv_t[:, L + kb * P:L + (kb + 1) * P],
                        sin_t[:, 0:L],
                        start=False, stop=(m == M_FREQ - 1),
                    )

            rc_ts = [None] * M_FREQ
            sin_ts = [None] * M_FREQ
            h_ts = [None] * M_FREQ

            def emit_red(m):
                # kneg = -round(wp*qkp) (bf16-exact integer);
                # rc = wp*qkp + kneg on the PE in fp32 PSUM.
                ub_t = ub_pool.tile([H, 2 * L], F32, name=f"ub{m}", tag="ub")
                nc.vector.tensor_scalar(
                    ub_t[:], qkp[:], float(WP[m]), RND_C,
                    mybir.AluOpType.mult, mybir.AluOpType.add,
                )
                kn_t = kn_pool.tile([H, 2 * L], BF16, name=f"kn{m}", tag="kn")
                nc.vector.tensor_scalar(
                    kn_t[:], ub_t[:], -1.0, RND_C,
                    mybir.AluOpType.mult, mybir.AluOpType.add,
                )
                rc_t = ps_pool.tile([H, 2 * L], F32, name=f"rc{m}", tag="ps")
                for half in range(2):
                    sl = slice(half * L, (half + 1) * L)
                    nc.tensor.matmul(
                        rc_t[:, sl], dgw[:, m - 1, :], qkp[:, sl],
                        start=True, stop=False,
                    )
                    nc.tensor.matmul(
                        rc_t[:, sl], ident[:], kn_t[:, sl],
                        start=False, stop=True,
                    )
                rc_ts[m] = rc_t

            def emit_act(m, split=False):
                s_h = PI if m > 0 else PI * float(WP[0])
                s_sin = TWO_PI if m > 0 else TWO_PI * float(WP[0])
                src = rc_ts[m] if m > 0 else qkp
                h_t = h_pool.tile([H, 2 * L], FP16, name=f"h{m}", tag="h")
                sin_t = sin_pool.tile([H, 2 * L], BF16, name=f"sin{m}",
                                      tag="sin")
                halves = ((0, L), (L, 2 * L)) if split else ((0, 2 * L),)
                for lo, hi in halves:
                    nc.scalar.activation(
                        h_t[:, lo:hi], src[:, lo:hi],
                        mybir.ActivationFunctionType.Sin, scale=s_h,
                    )
                for lo, hi in halves:
                    nc.scalar.activation(
                        sin_t[:, lo:hi], src[:, lo:hi],
                        mybir.ActivationFunctionType.Sin, scale=s_sin,
                    )
                sin_ts[m], h_ts[m] = sin_t, h_t

            def emit_tail(m):
                # cv = vb - 2 vb h^2  ( = vb*cos(w x) ), carrying v*beta.
                h2_t = h2_pool.tile([H, 2 * L], FP16, name=f"h2_{m}", tag="h2")
                nc.vector.tensor_tensor(
                    h2_t[:], h_ts[m][:], h_ts[m][:], mybir.AluOpType.mult
                )
                cv_t = cv_pool.tile([H, 2 * L], BF16, name=f"cv{m}", tag="cv")
                nc.vector.tensor_scalar(
                    cv_t[:], h2_t[:],
                    vb_sb[:, M_FREQ + m:M_FREQ + m + 1],
                    vb_sb[:, m:m + 1],
                    mybir.AluOpType.mult, mybir.AluOpType.add,
                )
                emit_scores(m, sin_ts[m], cv_t)

            # ACT: h_m then sin_m per m. DVE per cycle: [h2_m, cv_m,
            # ub_{m+1}, kneg_{m+1}] -- cv lands right after h2, the next
            # m's reduction follows.
            # rc1 AND rc2 matmuls go to the PE queue before the m0 score
            # matmuls: the m0 scores wait on the cv0 chain, and rc2 must
            # not sit behind them or ACT stalls at m2.
            emit_act(0, split=True)
            emit_red(1)
            emit_red(2)
            emit_act(1)
            emit_tail(0)
            emit_act(2)
            emit_red(3)
            emit_tail(1)
            emit_act(3)
            emit_tail(2)
            emit_tail(3)

            # ---------------- softmax + context (transposed) --------------
            sums_ps = ps_pool.tile([P, L], F32, tag="ps", name="sums_ps")
            ctx01 = ps_pool.tile([P, 2 * D], F32, tag="ps", name="ctx01")
            ctx23 = ps_pool.tile([P, 2 * D], F32, tag="ps", name="ctx23")
            ctx_slice = [(ctx01, 0), (ctx01, 1), (ctx23, 0), (ctx23, 1)]

            p01 = p_pool.tile([P, 2 * L], BF16, name="p01", tag="p")
            p23 = p_pool.tile([P, 2 * L], BF16, name="p23", tag="p")
            p_of = [(p01, 0), (p01, 1), (p23, 0), (p23, 1)]

            def emit_sums(kb):
                pt, half = p_of[kb]
                nc.tensor.matmul(
                    sums_ps[:], ones_sb[:],
                    pt[:, half * L:(half + 1) * L],
                    start=(kb == 0), stop=(kb == NQB - 1),
                )

            def emit_ctx(kb):
                pt, half = p_of[kb]
                for qb in range(NQB):
                    ct, chalf = ctx_slice[qb]
                    nc.tensor.matmul(
                        ct[:, chalf * D:(chalf + 1) * D],
                        pt[:, half * L + qb * P:half * L + (qb + 1) * P],
                        val_sb[:, kb, :],
                        start=(kb == 0), stop=(kb == NQB - 1),
                    )

            nc.scalar.activation(
                p01[:], score01[:], mybir.ActivationFunctionType.Exp,
                bias=neg4[:],
            )
            emit_sums(0)
            emit_sums(1)
            emit_ctx(0)
            emit_ctx(1)
            nc.scalar.activation(
                p23[:], score23[:], mybir.ActivationFunctionType.Exp,
                bias=neg4[:],
            )
            emit_sums(2)
            emit_sums(3)

            # Normalize without extra ACT table sets. sums rows are all
            # equal; thin transposes + narrow reciprocals give per-partition
            # inv (ctx scale); colrep + PE transpose rebuild the q-major
            # broadcast for the attn scale. The thin transposes are emitted
            # BEFORE the kb2/kb3 context matmuls so the reciprocal chain is
            # not stuck behind them on the PE queue.
            sums_sb = const.tile([P, L], BF16)
            nc.scalar.copy(sums_sb[:], sums_ps[:])  # ACT is idle post-exp
            sT_ps = ps_pool.tile([P, 2 * NQB], BF16, tag="ps", name="sT_ps")
            invT_sb = const.tile([P, NQB], F32)
            colrep = const.tile([P, L], BF16)
            invb_ps = ps_pool.tile([P, L], BF16, tag="ps", name="invb_ps")
            for qb in range(NQB):
                nc.tensor.transpose(
                    sT_ps[:, 2 * qb:2 * qb + 1],
                    sums_sb[0:1, qb * P:(qb + 1) * P],
                    ident[0:1, 0:1],
                )
            emit_ctx(2)
            emit_ctx(3)
            for qb in range(NQB):
                nc.vector.reciprocal(
                    invT_sb[:, qb:qb + 1], sT_ps[:, 2 * qb:2 * qb + 1]
                )
                nc.vector.tensor_scalar_mul(
                    colrep[:, qb * P:(qb + 1) * P], ones_sb[:],
                    invT_sb[:, qb:qb + 1],
                )
            for qb in range(NQB):
                nc.tensor.transpose(
                    invb_ps[:, qb * P:(qb + 1) * P],
                    colrep[:, qb * P:(qb + 1) * P],
                    ident[:],
                )
            inv_bc = const.tile([P, L], BF16)
            nc.vector.tensor_copy(inv_bc[:], invb_ps[:])

            attn_sbs = [
                out_pool.tile([P, 2, L], BF16, name=f"attn_sb{c}", tag="o")
                for c in range(2)
            ]
            ctx_sbs = [
                out_pool.tile([P, 2, D], BF16, name=f"ctx_sb{c}", tag="o")
                for c in range(2)
            ]
            for kb in range(NQB):
                pt, half = p_of[kb]
                nc.vector.tensor_tensor(
                    attn_sbs[kb // 2][:, kb % 2, :],
                    pt[:, half * L:(half + 1) * L], inv_bc[:],
                    mybir.AluOpType.mult,
                )
                if kb == 1:
                    nc.sync.dma_start(out=attn_d[0], in_=attn_sbs[0][:])
                if kb == 3:
                    nc.gpsimd.dma_start(out=attn_d[1], in_=attn_sbs[1][:])
            for qb in range(NQB):
                # ctx scales split across ACT (idle after exps) and DVE so
                # neither engine serializes the last outputs.
                ct, chalf = ctx_slice[qb]
                if qb < 2:
                    nc.scalar.mul(
                        ctx_sbs[qb // 2][:, qb % 2, :],
                        ct[:, chalf * D:(chalf + 1) * D],
                        invT_sb[:, qb:qb + 1],
                    )
                else:
                    nc.vector.tensor_scalar_mul(
                        ctx_sbs[qb // 2][:, qb % 2, :],
                        ct[:, chalf * D:(chalf + 1) * D],
                        invT_sb[:, qb:qb + 1],
                    )
                if qb == 1:
                    nc.scalar.dma_start(out=ctx_d[0], in_=ctx_sbs[0][:])
                if qb == 3:
                    nc.sync.dma_start(out=ctx_d[1], in_=ctx_sbs[1][:])

    nc.compile()
    return nc


def _get_nc():
    global _CACHED_NC
    if _CACHED_NC is None:
        _CACHED_NC = _build_nc()
    return _CACHED_NC


def _in_maps(query, key, value, w1, w2, v):
    import ml_dtypes as _md

    f = np.float32
    bf = _md.bfloat16

    def tile_rows(arr):
        # [R, C] with R = NB*P  ->  [P, NB, C]: partition-major, so each
        # SBUF partition's data is one contiguous DRAM line.
        r, c = arr.shape
        nb = r // P
        return np.ascontiguousarray(arr.reshape(nb, P, c).transpose(1, 0, 2))

    w1T = tile_rows(np.asarray(w1, dtype=f).T.astype(bf))   # [P, NDB, H]
    w2T = tile_rows(np.asarray(w2, dtype=f).T.astype(bf))
    vb1 = (np.asarray(v, dtype=np.float64)[0][:, None] * BETA[None, :])
    vbm = np.concatenate([vb1, -2.0 * vb1], axis=1).astype(f)  # [H, 2M]
    maps = []
    for b in range(B):
        qT = tile_rows(np.asarray(query[b], dtype=f).T.astype(bf))
        kT = tile_rows(np.asarray(key[b], dtype=f).T.astype(bf))
        maps.append(
            {
                "qw": np.ascontiguousarray(np.concatenate([qT, w1T], axis=2)),
                "kw": np.ascontiguousarray(np.concatenate([kT, w2T], axis=2)),
                "val": tile_rows(np.asarray(value[b], dtype=f).astype(bf)),
                "vb": vbm,
            }
        )
    return maps


def run(query, key, value, w1, w2, v, trace=False, **spmd_kwargs):
    nc = _get_nc()
    res = run_bass_kernel_spmd(
        nc,
        _in_maps(query, key, value, w1, w2, v),
        list(range(B)),
        trace=trace,
        **spmd_kwargs,
    )

    def unpack(arr):
        # [2, P, 2, L] pairs -> [512, 512] with rows (2c+j)*128 + p
        a = np.asarray(arr).astype(np.float32)
        return a.transpose(0, 2, 1, 3).reshape(L, L)

    attn = np.stack(
        [unpack(res.results[b]["attn"]).T for b in range(B)]
    )
    ctx = np.stack(
        [unpack(res.results[b]["ctx"]) for b in range(B)]
    )
    return (attn, ctx), res


def kernel(query, key, value, w1, w2, v):
    (attn, ctx), _ = run(query, key, value, w1, w2, v, trace=False)
    return (attn, ctx)
